# revision 16
# baseline (speedup 1.0000x reference)
"""Trainium2 Bass kernel for a channel co-attention module.

Math (per sample):
    x1f = x1 / ||x1||_row, x2f = x2 / ||x2||_row          (L2 over spatial)
    att = x1f @ x2f.T                                      [c1, c2]
    out1 = alpha * softmax_rows(att) @ x2 + x1
    out2 = beta  * softmax_rows(att.T) @ x1 + x2

Sharding: batch (n=32) split 4-per-core over 8 NeuronCores, pure data
parallel; alpha/beta replicated.

Design: raw x is cast to bf16 on load and kept resident (8 MiB/sample);
the gram runs on the raw bf16 data and the L2 normalization folds into
the softmax stage: E = exp(rn1_i * Graw_ij * rn2_j) via a broadcast-row
multiply (rn2) + per-partition activation scale (rn1).  The softmax
denominators fold into the output stage: out = (fake_psum * s_col) + x
with s_col = alpha/rs_i (fake1) or beta/cs_j (fake2), so fake2's matmul
weights are E itself and fake1's are transpose(E).
"""

import os
import sys

import numpy as np

if not os.path.isdir(os.path.join(sys.prefix, "concourse")):
    for _p in ("/opt/trn_rl_repo",):
        if os.path.isdir(_p) and _p not in sys.path:
            sys.path.append(_p)

import concourse.bacc as bacc
import concourse.bass as bass
import concourse.tile as tile
from concourse import mybir
from concourse.bass_utils import run_bass_kernel_spmd
from concourse.masks import make_identity

F32 = mybir.dt.float32
BF16 = mybir.dt.bfloat16
AF = mybir.ActivationFunctionType
ALU = mybir.AluOpType

N_FULL, C, H, W = 32, 512, 64, 64
HW = H * W                      # 4096
N_CORES = 8
NS = N_FULL // N_CORES          # samples per core
CT = C // 128                   # 4 channel tiles
LW = 2048                       # load width (1 MiB f32 DMAs)
LB = HW // LW                   # 2 load halves
SB = HW // 512                  # 8 spatial blocks of 512
NE = 512 // 128                 # 128-chunks per spatial block

LAST_RESULTS = None             # BassKernelResults of the most recent run


class Ctx:
    """Per-build shared state."""


def _emit_load_phase(nc, st, si, lo, hi):
    """Loads + squares + bf16 casts for sample si, tiles [lo, hi).

    DMA on sync queue, squares on ACT (with accum for norms), casts on
    GpSimd.  Tile index order is (lb, tn, t) so block 0's eight tiles
    arrive first and sample-(si) frees (lb-major in the fake phase)
    match allocation order.  The [lo, hi) split lets the caller
    interleave these loads with the previous sample's stores on the
    sync queue (required to avoid a pool-rotation deadlock).
    """
    x_d = (st.x1_d, st.x2_d)
    if si not in st.xb:
        st.xb[si] = [[[None] * LB for _ in range(CT)] for _ in range(2)]
        st.part[si] = [[None] * CT for _ in range(2)]
        for tn in range(2):
            for t in range(CT):
                st.part[si][tn][t] = st.small.tile(
                    [128, LB], F32, tag="part", bufs=18,
                    name=f"part{si}_{tn}_{t}")
    for j in range(lo, hi):
        lb, tn, t = j // 8, (j // 4) % 2, j % 4
        xb = st.xbp.tile([128, LW], BF16, tag="xb",
                         name=f"xb{si}_{tn}_{t}_{lb}")
        # gpsimd (SWDGE) DMAs can downconvert f32->bf16 in flight
        nc.gpsimd.dma_start(
            out=xb,
            in_=x_d[tn][si, t * 128:(t + 1) * 128,
                        lb * LW:(lb + 1) * LW],
        )
        st.xb[si][tn][t][lb] = xb


def _emit_sq_accum(nc, st, si):
    """Sum-of-squares accumulation from the bf16 tiles (DVE, fused).

    Emitted late (with the norm finalization) so these vector ops don't
    head-of-line-block the previous sample's output stage.
    """
    for j in range(16):
        lb, tn, t = j // 8, (j // 4) % 2, j % 4
        xb = st.xb[si][tn][t][lb]
        sq = st.sq.tile([128, LW], BF16, tag="sq", name=f"sq{si}")
        nc.scalar.activation(
            out=sq, in_=xb, func=AF.Square,
            accum_out=st.part[si][tn][t][:, lb:lb + 1],
        )


def _emit_norm_dve(nc, st, si):
    """Tiny per-sample norm finalization: rn = 1/sqrt(sum x^2).

    Emitted late so these DVE ops don't head-of-line-block the previous
    sample's output stage on the vector queue.
    """
    st.rn[si] = [[None] * CT for _ in range(2)]
    for tn in range(2):
        for t in range(CT):
            p = st.part[si][tn][t]
            nsq = st.small.tile([128, 1], F32, tag="nsq", bufs=4,
                                name=f"nsq{si}")
            nc.vector.reduce_sum(out=nsq, in_=p, axis=mybir.AxisListType.X)
            n_c = st.small.tile([128, 1], F32, tag="ncol", bufs=4,
                                name=f"ncol{si}")
            nc.scalar.sqrt(n_c, nsq)
            r_c = st.small.tile([128, 1], F32, tag="rncol", bufs=18,
                                name=f"rncol{si}_{tn}_{t}")
            nc.vector.reciprocal(r_c, n_c)
            st.rn[si][tn][t] = r_c


def _emit_tg_blocks(nc, st, si, blocks):
    """Transpose + gram accumulation for the given spatial blocks."""
    if si not in st.g_ps:
        st.g_ps[si] = [
            st.ps_g.tile([128, 512], F32, tag="g", bufs=CT,
                         name=f"g_ps{si}_{m}")
            for m in range(CT)
        ]
    g_ps = st.g_ps[si]
    for b in blocks:
        lb, off = b // 4, (b % 4) * 512
        for e in range(NE):
            ch_sb = []
            for tn in range(2):
                ch_ps = st.ps_t.tile([128, 512], BF16, tag="t",
                                     name=f"chps{si}_{tn}_{b}_{e}")
                for t in range(CT):
                    nc.tensor.transpose(
                        out=ch_ps[:, t * 128:(t + 1) * 128],
                        in_=st.xb[si][tn][t][lb][:, off + e * 128:
                                                 off + (e + 1) * 128],
                        identity=st.ident_b,
                    )
                c_sb = st.chk.tile([128, 512], BF16, tag="chk",
                                   name=f"chk{si}_{tn}_{b}_{e}")
                if tn == 0:
                    nc.scalar.copy(out=c_sb, in_=ch_ps)
                else:
                    nc.vector.tensor_copy(out=c_sb, in_=ch_ps)
                ch_sb.append(c_sb)
            first = (b == 0 and e == 0)
            last = (b == SB - 1 and e == NE - 1)
            for m in range(CT):
                nc.tensor.matmul(
                    g_ps[m],
                    lhsT=ch_sb[0][:, m * 128:(m + 1) * 128],
                    rhs=ch_sb[1],
                    start=first, stop=last,
                )


def _emit_softmax_p1(nc, st, si):
    """Normalization fold + exp: E = exp(rn1_i * Graw * rn2_j)."""
    # rn2 as a broadcast matrix: rn2_row via PE transposes of the per-tile
    # rn2 columns, then ones (x) rn2_row outer product.
    r2ps = st.ps_t.tile([1, 512], F32, tag="t", name=f"r2ps{si}")
    for t in range(CT):
        nc.tensor.transpose(
            out=r2ps[:, t * 128:(t + 1) * 128],
            in_=st.rn[si][1][t],
            identity=st.ident_f,
        )
    r2row = st.small.tile([1, 512], F32, tag="r2row", bufs=1,
                          name=f"r2row{si}")
    nc.scalar.copy(out=r2row, in_=r2ps)
    bc_ps = st.ps_f.tile([128, 512], F32, tag="f", name=f"bcps{si}")
    nc.tensor.matmul(bc_ps, lhsT=st.ones_row_f, rhs=r2row,
                     start=True, stop=True)
    bc_sb = st.small.tile([128, 512], F32, tag="bc", bufs=1,
                          name=f"bc{si}")
    nc.scalar.copy(out=bc_sb, in_=bc_ps)

    # E = exp(rn1_i * Graw * rn2_j), rs = row sums (fused accumulate)
    st.e_t[si], st.rs[si] = [], []
    for m in range(CT):
        gn = st.gn.tile([128, 512], BF16, tag="gn", bufs=3,
                        name=f"gn{si}_{m}")
        nc.vector.tensor_mul(gn, st.g_ps[si][m], bc_sb)
        e_m = st.e_p.tile([128, 512], BF16, tag="E", name=f"E{si}_{m}")
        rs_m = st.small.tile([128, 1], F32, tag="rs", bufs=5,
                             name=f"rs{si}_{m}")
        nc.scalar.activation(out=e_m, in_=gn, func=AF.Exp,
                             scale=st.rn[si][0][m], accum_out=rs_m)
        st.e_t[si].append(e_m)
        rc = st.small.tile([128, 1], F32, tag="rsr", bufs=5,
                           name=f"rsr{si}_{m}")
        nc.vector.reciprocal(rc, rs_m)
        as_m = st.small.tile([128, 1], F32, tag="as", bufs=5,
                             name=f"as{si}_{m}")
        nc.vector.tensor_mul(as_m, rc, st.alpha_sb)
        st.rs[si].append(as_m)
    del st.g_ps[si]  # psum banks recycle to the next sample's gram


def _emit_softmax_p2(nc, st, si):
    """Column sums cs_j = sum_i E_ij (PE accumulating ones-matmul)."""
    cs_ps = st.ps_f.tile([1, 512], F32, tag="f", name=f"cs{si}")
    for m in range(CT):
        nc.tensor.matmul(cs_ps, lhsT=st.ones_col_b, rhs=st.e_t[si][m],
                         start=(m == 0), stop=(m == CT - 1))
    csinv = st.small.tile([1, 512], F32, tag="csinv", bufs=1,
                          name=f"csinv{si}")
    nc.vector.reciprocal(csinv, cs_ps)
    st.csinv[si] = csinv


def _emit_softmax_p3(nc, st, si):
    """beta/cs as per-tile columns + AT = E.T (fake1 weights)."""
    csinv = st.csinv[si]
    cc_ps = st.ps_t.tile([128, CT], F32, tag="t", name=f"ccps{si}")
    for t in range(CT):
        nc.tensor.transpose(
            out=cc_ps[:, t:t + 1],
            in_=csinv[:, t * 128:(t + 1) * 128],
            identity=st.ident_f1,
        )
    bs_cols = st.small.tile([128, CT], F32, tag="bscols", bufs=2,
                            name=f"bscols{si}")
    nc.vector.tensor_scalar_mul(out=bs_cols, in0=cc_ps,
                                scalar1=st.beta_sb)
    st.bs[si] = bs_cols

    # AT[j, i] = E[i, j]  (fake1 weights, PE transpose)
    st.at_t[si] = []
    for t2 in range(CT):
        at_ps = st.ps_t.tile([128, 512], BF16, tag="t",
                             name=f"atps{si}_{t2}")
        for m in range(CT):
            nc.tensor.transpose(
                out=at_ps[:, m * 128:(m + 1) * 128],
                in_=st.e_t[si][m][:, t2 * 128:(t2 + 1) * 128],
                identity=st.ident_b,
            )
        at_m = st.at_sb.tile([128, 512], BF16, tag="AT",
                             name=f"AT{si}_{t2}")
        nc.vector.tensor_copy(out=at_m, in_=at_ps)
        st.at_t[si].append(at_m)


def _emit_fakes(nc, st, si, lb):
    """Fake matmuls + fused scale/residual + batched stores, one half."""
    o_d = (st.o1_d, st.o2_d)
    w_t = (st.at_t[si], st.e_t[si])
    for fk in range(2):
        rhs_tn = 1 - fk
        for m in range(CT):
            stg = st.stg.tile([128, LW], F32, tag="st",
                              name=f"st{si}_{lb}_{fk}_{m}")
            scale = (st.rs[si][m] if fk == 0
                     else st.bs[si][:, m:m + 1])
            for nbl in range(4):
                fp = st.ps_f.tile([128, 512], F32, tag="f",
                                  name=f"fp{si}_{fk}_{m}_{lb}_{nbl}")
                for k in range(CT):
                    nc.tensor.matmul(
                        fp,
                        lhsT=w_t[fk][k][:, m * 128:(m + 1) * 128],
                        rhs=st.xb[si][rhs_tn][k][lb][:, nbl * 512:
                                                     (nbl + 1) * 512],
                        start=(k == 0), stop=(k == CT - 1),
                    )
                nc.vector.scalar_tensor_tensor(
                    out=stg[:, nbl * 512:(nbl + 1) * 512],
                    in0=fp,
                    scalar=scale,
                    in1=st.xb[si][fk][m][lb][:, nbl * 512:
                                             (nbl + 1) * 512],
                    op0=ALU.mult,
                    op1=ALU.add,
                )
            nc.sync.dma_start(
                out=o_d[fk][si, m * 128:(m + 1) * 128,
                           lb * LW:(lb + 1) * LW],
                in_=stg,
            )


def build_kernel():
    nc = bacc.Bacc("TRN2", target_bir_lowering=False)
    st = Ctx()
    st.x1_d = nc.dram_tensor("x1", [NS, C, HW], F32, kind="ExternalInput")
    st.x2_d = nc.dram_tensor("x2", [NS, C, HW], F32, kind="ExternalInput")
    al_d = nc.dram_tensor("alpha", [1], F32, kind="ExternalInput")
    be_d = nc.dram_tensor("beta", [1], F32, kind="ExternalInput")
    st.o1_d = nc.dram_tensor("out1", [NS, C, HW], F32, kind="ExternalOutput")
    st.o2_d = nc.dram_tensor("out2", [NS, C, HW], F32, kind="ExternalOutput")

    with tile.TileContext(nc) as tc:
        with (
            tc.tile_pool(name="singles", bufs=1) as singles,
            tc.tile_pool(name="xbp", bufs=30) as xbp,
            tc.tile_pool(name="sq", bufs=2) as sq,
            tc.tile_pool(name="chk", bufs=4) as chk,
            tc.tile_pool(name="gn", bufs=3) as gn,
            tc.tile_pool(name="E", bufs=6) as e_p,
            tc.tile_pool(name="AT", bufs=6) as at_sb,
            tc.tile_pool(name="stg", bufs=4) as stg,
            tc.tile_pool(name="small", bufs=4) as small,
            tc.tile_pool(name="psG", bufs=1, space="PSUM") as ps_g,
            tc.tile_pool(name="psT", bufs=2, space="PSUM") as ps_t,
            tc.tile_pool(name="psF", bufs=2, space="PSUM") as ps_f,
        ):
            st.xbp, st.sq, st.chk, st.gn = xbp, sq, chk, gn
            st.e_p, st.at_sb, st.stg, st.small = e_p, at_sb, stg, small
            st.ps_g, st.ps_t, st.ps_f = ps_g, ps_t, ps_f

            st.ident_b = singles.tile([128, 128], BF16, name="ident_b")
            make_identity(nc, st.ident_b)
            st.ident_f = singles.tile([128, 128], F32, name="ident_f")
            make_identity(nc, st.ident_f)
            st.ident_f1 = singles.tile([1, 1], F32, name="ident_f1")
            nc.vector.memset(st.ident_f1, 1.0)
            st.ones_col_b = singles.tile([128, 1], BF16, name="ones_col_b")
            nc.vector.memset(st.ones_col_b, 1.0)
            st.ones_row_f = singles.tile([1, 128], F32, name="ones_row_f")
            nc.vector.memset(st.ones_row_f, 1.0)
            st.alpha_sb = singles.tile([128, 1], F32, name="alpha_sb")
            nc.gpsimd.dma_start(
                out=st.alpha_sb,
                in_=bass.AP(tensor=al_d, offset=0, ap=[[0, 128], [1, 1]]),
            )
            st.beta_sb = singles.tile([128, 1], F32, name="beta_sb")
            nc.gpsimd.dma_start(
                out=st.beta_sb,
                in_=bass.AP(tensor=be_d, offset=0, ap=[[0, 128], [1, 1]]),
            )

            st.xb, st.part, st.rn = {}, {}, {}
            st.g_ps, st.e_t, st.at_t = {}, {}, {}
            st.rs, st.bs, st.csinv = {}, {}, {}

            _emit_load_phase(nc, st, 0, 0, 16)
            _emit_sq_accum(nc, st, 0)
            _emit_norm_dve(nc, st, 0)
            _emit_tg_blocks(nc, st, 0, range(SB))
            for si in range(NS):
                if si + 1 < NS:
                    _emit_load_phase(nc, st, si + 1, 0, 10)
                _emit_softmax_p1(nc, st, si)
                if si + 1 < NS:
                    _emit_tg_blocks(nc, st, si + 1, [0])
                _emit_softmax_p2(nc, st, si)
                if si + 1 < NS:
                    _emit_tg_blocks(nc, st, si + 1, [1])
                _emit_softmax_p3(nc, st, si)
                _emit_fakes(nc, st, si, 0)
                if si + 1 < NS:
                    _emit_load_phase(nc, st, si + 1, 10, 16)
                _emit_fakes(nc, st, si, 1)
                if si + 1 < NS:
                    _emit_sq_accum(nc, st, si + 1)
                    _emit_norm_dve(nc, st, si + 1)
                    _emit_tg_blocks(nc, st, si + 1, range(2, SB))
    if not nc.is_finalized():
        nc.finalize()
    return nc


_NC_CACHE = None


def kernel(x1, x2, alpha, beta):
    global _NC_CACHE, LAST_RESULTS
    x1 = np.ascontiguousarray(np.asarray(x1, dtype=np.float32))
    x2 = np.ascontiguousarray(np.asarray(x2, dtype=np.float32))
    alpha = np.ascontiguousarray(np.asarray(alpha, dtype=np.float32))
    beta = np.ascontiguousarray(np.asarray(beta, dtype=np.float32))
    n, c, h, w = x1.shape
    assert (n, c, h * w) == (N_FULL, C, HW)

    if _NC_CACHE is None:
        _NC_CACHE = build_kernel()
    nc = _NC_CACHE

    in_maps = []
    for core in range(N_CORES):
        s = slice(core * NS, (core + 1) * NS)
        in_maps.append({
            "x1": x1[s].reshape(NS, C, HW),
            "x2": x2[s].reshape(NS, C, HW),
            "alpha": alpha,
            "beta": beta,
        })

    res = run_bass_kernel_spmd(nc, in_maps, core_ids=list(range(N_CORES)))
    LAST_RESULTS = res
    out1 = np.concatenate([r["out1"] for r in res.results], axis=0)
    out2 = np.concatenate([r["out2"] for r in res.results], axis=0)
    return (out1.reshape(n, c, h, w).astype(np.float32),
            out2.reshape(n, c, h, w).astype(np.float32))


if __name__ == "__main__":
    rng = np.random.default_rng(0)
    x1 = rng.standard_normal((N_FULL, C, H, W), dtype=np.float32)
    x2 = rng.standard_normal((N_FULL, C, H, W), dtype=np.float32)
    alpha = np.full((1,), 0.37, np.float32)
    beta = np.full((1,), -0.21, np.float32)
    o1, o2 = kernel(x1, x2, alpha, beta)
    # cpu reference
    x1d = x1.reshape(N_FULL, C, HW).astype(np.float64)
    x2d = x2.reshape(N_FULL, C, HW).astype(np.float64)
    x1f = x1d / np.linalg.norm(x1d, axis=2, keepdims=True)
    x2f = x2d / np.linalg.norm(x2d, axis=2, keepdims=True)
    att = np.einsum('nis,njs->nij', x1f, x2f)
    ea = np.exp(att - att.max(axis=2, keepdims=True))
    a1 = ea / ea.sum(axis=2, keepdims=True)
    f1 = np.einsum('nij,njs->nis', a1, x2d)
    eat = np.exp(att - att.max(axis=1, keepdims=True))
    a2 = eat / eat.sum(axis=1, keepdims=True)
    f2 = np.einsum('nij,nis->njs', a2, x1d)
    r1 = (0.37 * f1 + x1d).reshape(N_FULL, C, H, W)
    r2 = (-0.21 * f2 + x2d).reshape(N_FULL, C, H, W)
    for name, a, b in (("out1", o1, r1), ("out2", o2, r2)):
        nr = np.linalg.norm(a - b) / np.linalg.norm(b)
        print(f"{name}: norm_rel={nr:.3e}")
